# revision 1
# baseline (speedup 1.0000x reference)
"""Balanced dice loss (histogram binning) on 8 Trainium2 NeuronCores.

Math: with t ∈ {0,1} and p = sigmoid(x), the reference loss only needs
four global sums:
    S_t   = Σ t            (count of ones — the bincount)
    S_pt  = Σ p·t
    S_pp  = Σ p²
    S_ppt = Σ p²·t
Then with c1 = S_t, c0 = N − c1, w0 = 1/(c0+s)², w1 = 1/(c1+s)²:
    intersection = w1·S_pt
    denominator  = w0·(S_pp − S_ppt) + w1·(S_ppt + c1)
    dice = 1 − (2·I + s)/(D + s)

Device kernel (data-parallel over 8 cores, batch-sharded), per [128,F] tile:
    ACT : p = sigmoid(x) (bf16 out); p² with row-accum → S_pp;
          float(t) copy on the first FA columns with row-accum → S_t part 1
    DVE : u = p·t (int32 converts in-pipe), w = u·p (= p²·t, bf16 2x mode);
          int32 row-reduce of t on the remaining columns → S_t part 2
    PE  : ones[128,128] @ chunks of u and w → PSUM column-sum accumulation
          (each chain alternates two PSUM banks; S_pt, S_ppt)
The work is split so ACT/DVE/PE all sit just under the ~94 µs DMA
roofline (32 MB/core at ~358 GB/s per-core HBM bandwidth).
Per-partition/per-tile partials are DMA'd out; host reduces in float64.
"""

import numpy as np

import concourse.bacc as bacc
import concourse.mybir as mybir
from concourse.bass_utils import run_bass_kernel_spmd
from concourse.tile import TileContext

N_CORES = 8
P = 128
TOTAL = 32 * 1024 * 1024  # elements in the full problem
PER_CORE = TOTAL // N_CORES  # 4,194,304
FREE = PER_CORE // P  # 32,768 f32 per partition
F = 4096  # tile free-dim
NT = FREE // F  # tiles per core
MMN = 512  # matmul moving free-dim (one PSUM bank)
FA = 2048  # S_t split: first FA columns summed on ACT, rest on DVE
NCH = F // MMN  # matmul chunks per tile
SMOOTH = 1e-05

_nc_cache = None


def _build_bass():
    nc = bacc.Bacc(None, target_bir_lowering=False)
    x = nc.dram_tensor("input", [P, FREE], mybir.dt.float32, kind="ExternalInput")
    t = nc.dram_tensor("target", [P, FREE], mybir.dt.int32, kind="ExternalInput")
    o_pt = nc.dram_tensor("o_pt", [1, 2 * MMN], mybir.dt.float32, kind="ExternalOutput")
    o_ppt = nc.dram_tensor("o_ppt", [1, 2 * MMN], mybir.dt.float32, kind="ExternalOutput")
    o_pp = nc.dram_tensor("o_pp", [P, NT], mybir.dt.float32, kind="ExternalOutput")
    o_t = nc.dram_tensor("o_t", [P, NT], mybir.dt.float32, kind="ExternalOutput")
    o_t2 = nc.dram_tensor("o_t2", [P, NT], mybir.dt.int32, kind="ExternalOutput")

    with TileContext(nc) as tc:
        with (
            tc.tile_pool(name="work", bufs=2) as pool,
            tc.tile_pool(name="stats", bufs=1) as spool,
            tc.tile_pool(name="ps", bufs=1, space="PSUM") as psum,
        ):
            s_t = spool.tile([P, NT], mybir.dt.float32)
            s_t2 = spool.tile([P, NT], mybir.dt.int32)
            s_pp = spool.tile([P, NT], mybir.dt.float32)
            junk = spool.tile([P, F], mybir.dt.float32, tag="junk")
            ones = spool.tile([P, P], mybir.dt.bfloat16, tag="ones")
            ps_pt_a = psum.tile([P, MMN], mybir.dt.float32, tag="ps_pt_a")
            ps_pt_b = psum.tile([P, MMN], mybir.dt.float32, tag="ps_pt_b")
            ps_ppt_a = psum.tile([P, MMN], mybir.dt.float32, tag="ps_ppt_a")
            ps_ppt_b = psum.tile([P, MMN], mybir.dt.float32, tag="ps_ppt_b")
            nc.any.memset(ones, 1.0)

            for i in range(NT):
                xt = pool.tile([P, F], mybir.dt.float32, tag="xt", bufs=3)
                tt = pool.tile([P, F], mybir.dt.int32, tag="tt", bufs=3)
                pt_ = pool.tile([P, F], mybir.dt.bfloat16, tag="p")
                u = pool.tile([P, F], mybir.dt.bfloat16, tag="u")
                w = pool.tile([P, F], mybir.dt.bfloat16, tag="w")

                nc.sync.dma_start(xt[:], x[:, i * F : (i + 1) * F])
                nc.sync.dma_start(tt[:], t[:, i * F : (i + 1) * F])

                # p = sigmoid(x)                                   [ACT]
                nc.scalar.activation(
                    pt_[:], xt[:], mybir.ActivationFunctionType.Sigmoid
                )
                # u = p·t (bf16·int32), w = u·p                    [DVE]
                nc.vector.tensor_tensor(
                    out=u[:], in0=pt_[:], in1=tt[:], op=mybir.AluOpType.mult
                )
                nc.vector.tensor_tensor(
                    out=w[:], in0=u[:], in1=pt_[:], op=mybir.AluOpType.mult
                )
                # S_pp row-accum via p² (junk sink)                [ACT]
                nc.scalar.activation(
                    junk[:],
                    pt_[:],
                    mybir.ActivationFunctionType.Square,
                    accum_out=s_pp[:, i : i + 1],
                )
                # S_t split: float(t) copy+accum on [:FA]          [ACT]
                nc.scalar.activation(
                    junk[:, :FA],
                    tt[:, :FA],
                    mybir.ActivationFunctionType.Copy,
                    accum_out=s_t[:, i : i + 1],
                )
                # ... and int32 row-reduce on [FA:] (exact)         [DVE]
                with nc.allow_low_precision("int32 add is exact"):
                    nc.vector.tensor_reduce(
                        s_t2[:, i : i + 1],
                        tt[:, FA:],
                        axis=mybir.AxisListType.X,
                        op=mybir.AluOpType.add,
                    )
                # column-sum accumulation of u and w; each chain
                # alternates two PSUM banks to pipeline the RMW    [PE]
                for s_, banks in ((u, (ps_pt_a, ps_pt_b)), (w, (ps_ppt_a, ps_ppt_b))):
                    for j in range(NCH):
                        nc.tensor.matmul(
                            banks[j % 2][:],
                            ones[:],
                            s_[:, j * MMN : (j + 1) * MMN],
                            start=(i == 0 and j < 2),
                            stop=(i == NT - 1 and j >= NCH - 2),
                        )

            fin = spool.tile([1, 4 * MMN], mybir.dt.float32, tag="fin")
            nc.vector.tensor_copy(fin[:, 0:MMN], ps_pt_a[0:1, :])
            nc.vector.tensor_copy(fin[:, MMN : 2 * MMN], ps_pt_b[0:1, :])
            nc.vector.tensor_copy(fin[:, 2 * MMN : 3 * MMN], ps_ppt_a[0:1, :])
            nc.vector.tensor_copy(fin[:, 3 * MMN :], ps_ppt_b[0:1, :])
            nc.sync.dma_start(o_pt[:], fin[:, 0 : 2 * MMN])
            nc.sync.dma_start(o_ppt[:], fin[:, 2 * MMN :])
            nc.sync.dma_start(o_pp[:], s_pp[:])
            nc.sync.dma_start(o_t[:], s_t[:])
            nc.sync.dma_start(o_t2[:], s_t2[:])
    nc.finalize()
    return nc


def _get_nc():
    global _nc_cache
    if _nc_cache is None:
        _nc_cache = _build_bass()
    return _nc_cache


def kernel(input, target, _trace=False):
    x = np.ascontiguousarray(np.asarray(input, dtype=np.float32)).reshape(
        N_CORES, P, FREE
    )
    t = np.ascontiguousarray(np.asarray(target, dtype=np.int32)).reshape(
        N_CORES, P, FREE
    )
    in_maps = [{"input": x[i], "target": t[i]} for i in range(N_CORES)]

    nc = _get_nc()
    res = run_bass_kernel_spmd(
        nc, in_maps, core_ids=list(range(N_CORES)), trace=_trace
    )
    kernel.last_results = res

    s_pt = s_ppt = s_pp = s_t = 0.0
    for r in res.results:
        s_pt += float(r["o_pt"].astype(np.float64).sum())
        s_ppt += float(r["o_ppt"].astype(np.float64).sum())
        s_pp += float(r["o_pp"].astype(np.float64).sum())
        s_t += float(r["o_t"].astype(np.float64).sum())
        s_t += float(r["o_t2"].astype(np.int64).sum())

    c1 = float(s_t)
    c0 = float(TOTAL - s_t)
    w0 = 1.0 / (c0 + SMOOTH) ** 2
    w1 = 1.0 / (c1 + SMOOTH) ** 2
    intersection = w1 * s_pt
    denominator = w0 * (s_pp - s_ppt) + w1 * (s_ppt + c1)
    dice = 1.0 - (2.0 * intersection + SMOOTH) / (denominator + SMOOTH)
    return np.asarray(dice, dtype=np.float32)



# revision 3
# speedup vs baseline: 1.0227x; 1.0227x over previous
"""Balanced dice loss (histogram binning) on 8 Trainium2 NeuronCores.

Math: with t ∈ {0,1} and p = sigmoid(x), the reference loss only needs
four global sums:
    S_t   = Σ t            (count of ones — the bincount)
    S_pt  = Σ p·t
    S_pp  = Σ p²
    S_ppt = Σ p²·t
Then with c1 = S_t, c0 = N − c1, w0 = 1/(c0+s)², w1 = 1/(c1+s)²:
    intersection = w1·S_pt
    denominator  = w0·(S_pp − S_ppt) + w1·(S_ppt + c1)
    dice = 1 − (2·I + s)/(D + s)

Since t² = t, with u = p·t: S_pt = Σu and S_ppt = Σu². The engine
dataflow is strictly one-way (ACT → DVE → PE) so no engine ever waits
on a consumer — this is what finally removes the cross-engine
ping-pong stalls. Per [128,c] tile:
    ACT : sigmoid(x) → p (bf16); Copy(t) accum → S_t (sink output)
    DVE : scalar_tensor_tensor u = p·t, accum → S_pt (fused row-sum);
          tensor_tensor sq = p·p (2x); tensor_tensor u ← u·u in-place
          (2x) so no extra SBUF buffer is needed for u²
    PE  : ones[128,1] @ chunks of sq → PSUM banks 0-1 → S_pp
          ones[128,1] @ chunks of u² → PSUM banks 2-3 → S_ppt
Per-tile engine cost sits under the DMA cadence (~430 GB/s), so the
kernel tracks the ~80µs DMA roofline. sq is computed before the
t-gated u so DVE stays busy while t is in flight. Tile sizes taper at
both ends to shorten fill and drain, and the two trailing tiny tiles
own dedicated DMA buffers so their loads are never gated on the shared
buffer rotation (that gating intermittently added 15-20µs of DMA
tail). Host reduces the per-partition/per-bank partials in float64.
"""

import numpy as np

import concourse.bacc as bacc
import concourse.mybir as mybir
from concourse.bass_utils import run_bass_kernel_spmd
from concourse.tile import TileContext

N_CORES = 8
P = 128
TOTAL = 32 * 1024 * 1024  # elements in the full problem
PER_CORE = TOTAL // N_CORES  # 4,194,304
FREE = PER_CORE // P  # 32,768 f32 per partition
TILES = [2048, 5632, 5632, 5632, 5632, 5120, 2048, 1024]  # sums to FREE
NDED = 2  # trailing tiles with dedicated DMA buffers (no rotation gating)
NT = len(TILES)
MMN = 512  # matmul moving free-dim (one PSUM bank row)
NBANK = 2  # PSUM banks per column-sum chain (sq and w each get 2)
NCHT = sum(c // MMN for c in TILES)  # matmul chunks per chain
SMOOTH = 1e-05

_nc_cache = None


def _build_bass():
    assert sum(TILES) == FREE
    nc = bacc.Bacc(None, target_bir_lowering=False)
    x = nc.dram_tensor("input", [P, FREE], mybir.dt.float32, kind="ExternalInput")
    t = nc.dram_tensor("target", [P, FREE], mybir.dt.int32, kind="ExternalInput")
    o_pp = nc.dram_tensor(
        "o_pp", [1, NBANK * MMN], mybir.dt.float32, kind="ExternalOutput"
    )
    o_ppt = nc.dram_tensor(
        "o_ppt", [1, NBANK * MMN], mybir.dt.float32, kind="ExternalOutput"
    )
    o_pt = nc.dram_tensor("o_pt", [P, NT], mybir.dt.float32, kind="ExternalOutput")
    o_ta = nc.dram_tensor("o_ta", [P, NT], mybir.dt.float32, kind="ExternalOutput")

    with TileContext(nc) as tc:
        with (
            tc.tile_pool(name="work", bufs=2) as pool,
            tc.tile_pool(name="stats", bufs=1) as spool,
            tc.tile_pool(name="ps", bufs=1, space="PSUM") as psum,
        ):
            s_pt = spool.tile([P, NT], mybir.dt.float32)
            s_ta = spool.tile([P, NT], mybir.dt.float32)
            sink = spool.tile([P, 1], mybir.dt.bfloat16, tag="sink")
            ones = spool.tile([P, 1], mybir.dt.bfloat16, tag="ones")
            banks = []
            for b in range(2 * NBANK):
                bank = psum.tile([1, MMN], mybir.dt.float32, tag=f"ps{b}")
                banks.append(bank)
            nc.any.memset(ones, 1.0)

            off = 0
            k = 0  # global PSUM chunk counter (per chain)
            for i, c in enumerate(TILES):
                pt_ = pool.tile([P, c], mybir.dt.bfloat16, tag="p")
                u = pool.tile([P, c], mybir.dt.bfloat16, tag="u")
                sq = pool.tile([P, c], mybir.dt.bfloat16, tag="sq")

                if i >= NT - NDED:
                    # dedicated buffers for the trailing tiny tiles: with
                    # the shared 2-buffer rotation their DMA cannot start
                    # until a sigmoid ~80µs in frees a buffer, which
                    # intermittently stretches the DMA tail by 15-20µs
                    xt = spool.tile([P, c], mybir.dt.float32, tag=f"xtl{i}")
                    tt = spool.tile([P, c], mybir.dt.int32, tag=f"ttl{i}")
                else:
                    xt = pool.tile([P, c], mybir.dt.float32, tag="xt")
                    tt = pool.tile([P, c], mybir.dt.int32, tag="tt")

                nc.sync.dma_start(xt[:], x[:, off : off + c])
                nc.sync.dma_start(tt[:], t[:, off : off + c])

                # p = sigmoid(x)                                   [ACT]
                nc.scalar.activation(
                    pt_[:], xt[:], mybir.ActivationFunctionType.Sigmoid
                )
                # float(t) copy (sink), accum → S_t                [ACT]
                nc.scalar.activation(
                    sink.broadcast_to([P, c]),
                    tt[:],
                    mybir.ActivationFunctionType.Copy,
                    accum_out=s_ta[:, i : i + 1],
                )
                # sq = p² in 2x mode (needs only p — keeps DVE
                # busy while this tile's t is still in flight)     [DVE]
                nc.vector.tensor_tensor(
                    out=sq[:], in0=pt_[:], in1=pt_[:], op=mybir.AluOpType.mult
                )
                # u = p·t (int32 converts in-pipe), accum → S_pt   [DVE]
                nc.vector.scalar_tensor_tensor(
                    out=u[:],
                    in0=pt_[:],
                    scalar=1.0,
                    in1=tt[:],
                    op0=mybir.AluOpType.mult,
                    op1=mybir.AluOpType.mult,
                    accum_out=s_pt[:, i : i + 1],
                )
                # u ← u² in-place, 2x mode (= p²t since t²=t)      [DVE]
                nc.vector.tensor_tensor(
                    out=u[:], in0=u[:], in1=u[:], op=mybir.AluOpType.mult
                )
                # column sums: sq → banks 0-1 (S_pp),
                #              u² → banks 2-3 (S_ppt)              [PE]
                for j in range(c // MMN):
                    nc.tensor.matmul(
                        banks[k % NBANK][:],
                        ones[:],
                        sq[:, j * MMN : (j + 1) * MMN],
                        start=(k < NBANK),
                        stop=(k >= NCHT - NBANK),
                    )
                    k += 1
                k -= c // MMN
                for j in range(c // MMN):
                    nc.tensor.matmul(
                        banks[NBANK + k % NBANK][:],
                        ones[:],
                        u[:, j * MMN : (j + 1) * MMN],
                        start=(k < NBANK),
                        stop=(k >= NCHT - NBANK),
                    )
                    k += 1
                off += c

            nc.sync.dma_start(o_pt[:], s_pt[:])
            nc.sync.dma_start(o_ta[:], s_ta[:])
            fin = spool.tile([1, 2 * NBANK * MMN], mybir.dt.float32, tag="fin")
            for b in range(2 * NBANK):
                dst = fin[:, b * MMN : (b + 1) * MMN]
                if b % 2 == 0:
                    nc.vector.tensor_copy(dst, banks[b][:])
                else:
                    nc.scalar.activation(
                        dst, banks[b][:], mybir.ActivationFunctionType.Copy
                    )
            nc.sync.dma_start(o_pp[:], fin[:, 0 : NBANK * MMN])
            nc.sync.dma_start(o_ppt[:], fin[:, NBANK * MMN :])
    nc.finalize()
    return nc


def _get_nc():
    global _nc_cache
    if _nc_cache is None:
        _nc_cache = _build_bass()
    return _nc_cache


def kernel(input, target, _trace=False):
    x = np.ascontiguousarray(np.asarray(input, dtype=np.float32)).reshape(
        N_CORES, P, FREE
    )
    t = np.ascontiguousarray(np.asarray(target, dtype=np.int32)).reshape(
        N_CORES, P, FREE
    )
    in_maps = [{"input": x[i], "target": t[i]} for i in range(N_CORES)]

    nc = _get_nc()
    res = run_bass_kernel_spmd(
        nc, in_maps, core_ids=list(range(N_CORES)), trace=_trace
    )
    kernel.last_results = res

    s_pt = s_ppt = s_pp = s_t = 0.0
    for r in res.results:
        s_pt += float(r["o_pt"].astype(np.float64).sum())
        s_ppt += float(r["o_ppt"].astype(np.float64).sum())
        s_pp += float(r["o_pp"].astype(np.float64).sum())
        s_t += float(r["o_ta"].astype(np.float64).sum())

    c1 = float(s_t)
    c0 = float(TOTAL - s_t)
    w0 = 1.0 / (c0 + SMOOTH) ** 2
    w1 = 1.0 / (c1 + SMOOTH) ** 2
    intersection = w1 * s_pt
    denominator = w0 * (s_pp - s_ppt) + w1 * (s_ppt + c1)
    dice = 1.0 - (2.0 * intersection + SMOOTH) / (denominator + SMOOTH)
    return np.asarray(dice, dtype=np.float32)
